# revision 22
# baseline (speedup 1.0000x reference)
"""DigitCaps dynamic-routing kernel for 8 Trainium2 NeuronCores.

Strategy: shard num_route_nodes (R=2048 -> 256/core); AllReduce the s
contraction. No u staging: u is recomputed on the PE each routing pass
(w re-DMA in fp16 is cheaper than staging u to DRAM, and the PE is
otherwise idle).

  - Iteration 1 (uniform coefficients): s1 = sum_r u_r accumulated
    entirely on the PE via route-pair-packed K=128 matmuls. No vector
    work at all.
  - Iterations i>=2: per r-tile, recompute u (K=64 matmuls, fp16 in,
    fp32 PSUM), ACT copies PSUM->SBUF fp16. Routing logits use the
    linearity trick  b_i = dot(u, v_1+...+v_{i-1})  so there is no
    persistent logit tensor. All heavy vector ops run on the DVE in
    fp16 2x mode: products as tensor_tensor, reductions as binary
    add-trees (tensor_reduce is 1x-only; trees are 2x => ~2x faster).
  - CM layout is m-major (cm = m*C + c) so every broadcast/tree view
    has unit innermost stride (2x eligible): dot tree halves m, the
    coefficient broadcast repeats over m with c contiguous.

Host-side prep is layout/dtype only (transpose + fp16 cast); all FLOPs
on device.
"""

import os
import sys

if "/opt/trn_rl_repo" not in sys.path:
    sys.path.insert(0, "/opt/trn_rl_repo")

import numpy as np

B, R, K, C, M = 128, 2048, 64, 32, 32
CM = C * M
N_CORES = 8
R_LOC = R // N_CORES
RT = int(os.environ.get("DC_RT", "16"))     # routes per tile (even)
SIM_MODE = os.environ.get("DC_SIM", "0") == "1"   # 1-core, collective->copy
# Ablation knob (timing experiments only; wrong numerics except "full"):
#   full | nosm (skip softmax) | nodot (skip p/tree/softmax) |
#   noq (skip q/s-tree) | prod (u production only) | fakear (no collective)
ABL = os.environ.get("DC_ABL", "full")
# If >0: route every GPS-th tile's q-mult to GPSIMD (offload experiment)
GPS = int(os.environ.get("DC_GPS", "0"))
# Split each routing iteration's AllReduce in two so the first half's
# collective overlaps the second half's compute.
SPLIT_AR = os.environ.get("DC_SPLITAR", "0") == "1"
# PSUM->SBUF u-copy engine: act | split | dve
CPENG = os.environ.get("DC_CPENG", "act")

_compiled = {}
LAST_RESULT = None


def _view(ap, dims):
    """Free-dim view of an AP: keep its partition dim, replace free dims by
    [step, count] pairs (element steps). step 0 = broadcast."""
    import concourse.bass as bass

    return bass.AP(
        tensor=ap.tensor,
        offset=ap.offset,
        ap=[list(ap.ap[0])] + [[s, c] for s, c in dims],
    )


def _ap(ap, dims):
    """Fully custom AP (all dims given) at the base offset of `ap`."""
    import concourse.bass as bass

    return bass.AP(
        tensor=ap.tensor,
        offset=ap.offset,
        ap=[[s, c] for s, c in dims],
    )


def _squash(nc, pool, s_ap, v_ap):
    """v = s * |s|^2 / ((1 + |s|^2) (sqrt(|s|^2) + 1e-8)), norm over m.
    m-major layout: sq[b,c] reduces view [(1,C),(C,M)]."""
    import concourse.mybir as mybir

    f32 = mybir.dt.float32
    op = mybir.AluOpType
    sq_full = pool.tile([B, CM], f32, tag="sq_full")
    nc.vector.tensor_tensor(sq_full[:], s_ap, s_ap, op=op.mult)
    sq = pool.tile([B, C], f32, tag="sq")
    nc.vector.tensor_reduce(
        sq[:], _view(sq_full[:], [(1, C), (C, M)]), axis=mybir.AxisListType.X,
        op=op.add)
    # sqrt(x) = exp(0.5*ln(x)): stays on the natural_log_exp ACT table set
    # (the Sqrt set would force a ~2.7us table swap around every iteration's
    # exp calls). ln(0)=-inf -> exp(-inf)=0, matching sqrt(0).
    rt = pool.tile([B, C], f32, tag="rt")
    nc.scalar.activation(rt[:], sq[:], mybir.ActivationFunctionType.Ln)
    nc.scalar.activation(rt[:], rt[:], mybir.ActivationFunctionType.Exp,
                         scale=0.5)
    nc.vector.tensor_scalar(rt[:], rt[:], 1e-8, None, op0=op.add)
    den = pool.tile([B, C], f32, tag="den")
    nc.vector.tensor_scalar(den[:], sq[:], 1.0, None, op0=op.add)
    nc.vector.tensor_tensor(den[:], den[:], rt[:], op=op.mult)
    fi = pool.tile([B, C], f32, tag="fi")
    nc.vector.reciprocal(fi[:], den[:])
    nc.vector.tensor_tensor(fi[:], fi[:], sq[:], op=op.mult)
    nc.vector.tensor_tensor(
        v_ap,
        _view(s_ap, [(C, M), (1, C)]),
        _view(fi[:], [(0, M), (1, C)]),
        op=op.mult,
    )


def _build(n_iters, repeat=1):
    import concourse.mybir as mybir
    import concourse.tile as tile
    from concourse import bacc

    f32 = mybir.dt.float32
    f16 = mybir.dt.float16
    op = mybir.AluOpType
    AX = mybir.AxisListType
    HALF = RT // 2
    NT = R_LOC // RT

    nc = bacc.Bacc("TRN2", target_bir_lowering=False, debug=False,
                   num_devices=1 if SIM_MODE else N_CORES)
    xT = nc.dram_tensor("xT", [R_LOC // 2, 2, K, B], f16,
                        kind="ExternalInput").ap()
    wT = nc.dram_tensor("wT", [R_LOC // 2, 2, K, CM], f16,
                        kind="ExternalInput").ap()
    out = nc.dram_tensor("out", [B, CM], f32, kind="ExternalOutput").ap()

    with tile.TileContext(nc) as tc:
        with (
            tc.tile_pool(name="sm", bufs=2) as sm,
            tc.tile_pool(name="persist", bufs=1) as persist,
            tc.tile_pool(name="xp", bufs=3) as xp,
            tc.tile_pool(name="wp", bufs=2) as wp,
            tc.tile_pool(name="up", bufs=2) as up,
            tc.tile_pool(name="pq", bufs=1) as pq,
            tc.tile_pool(name="pp", bufs=3, space="PSUM") as pp,
            tc.tile_pool(name="s1p", bufs=1, space="PSUM") as s1p,
            tc.tile_pool(name="drbounce", bufs=min(2 * n_iters * repeat, 8),
                         space="DRAM") as drb,
        ):
            v_sb = persist.tile([B, CM], f32)        # current v (output)
            vbar32 = persist.tile([B, CM], f32)      # sum of v's (logit trick)
            vbar16 = persist.tile([B, CM], f16)

            def dma_xw(t):
                """DMA one tile of x/w (RT routes = HALF pairs), packed
                [2K, HALF*B] / [2K, HALF*CM] (pair on partitions)."""
                xt = xp.tile([2 * K, HALF * B], f16)
                nc.sync.dma_start(
                    xt[:],
                    _ap(xT[t * HALF:(t + 1) * HALF],
                        [(B, 2 * K), (2 * K * B, HALF), (1, B)]))
                wt = wp.tile([2 * K, HALF * CM], f16)
                nc.sync.dma_start(
                    wt[:],
                    _ap(wT[t * HALF:(t + 1) * HALF],
                        [(CM, 2 * K), (2 * K * CM, HALF), (1, CM)]))
                return xt, wt

            def ar_start(s_acc_tile, tag):
                bin_ = drb.tile([B, CM], f32, tag=f"bin{tag}", name=f"bin{tag}")
                bout = drb.tile([B, CM], f32, tag=f"bout{tag}", name=f"bout{tag}")
                nc.sync.dma_start(bin_[:], s_acc_tile[:])
                if SIM_MODE or ABL == "fakear":
                    nc.sync.dma_start(bout[:], bin_[:])
                else:
                    nc.gpsimd.collective_compute(
                        "AllReduce", op.add,
                        replica_groups=[list(range(N_CORES))],
                        ins=[bin_.opt()], outs=[bout.opt()],
                    )
                return bout

            def allreduce_squash(s_accs, scale, first):
                bouts = [ar_start(s, i) for i, s in enumerate(s_accs)]
                s_sb = sm.tile([B, CM], f32, tag="s_sb")
                nc.sync.dma_start(s_sb[:], bouts[0][:])
                for bout in bouts[1:]:
                    more = sm.tile([B, CM], f32, tag="s_sb2")
                    nc.sync.dma_start(more[:], bout[:])
                    nc.vector.tensor_tensor(s_sb[:], s_sb[:], more[:],
                                            op=op.add)
                if scale != 1.0:
                    nc.vector.tensor_scalar(s_sb[:], s_sb[:], scale, None,
                                            op0=op.mult)
                _squash(nc, sm, s_sb[:], v_sb[:])
                if first:
                    nc.vector.tensor_copy(vbar32[:], v_sb[:])
                else:
                    nc.vector.tensor_tensor(vbar32[:], vbar32[:], v_sb[:],
                                            op=op.add)
                nc.vector.tensor_copy(vbar16[:], vbar32[:])

            def emit_iter1():
                """s1 = sum_r u_r entirely on PE: packed K=128 matmuls."""
                s1 = s1p.tile([B, CM], f32, name="s1")
                for t in range(NT):
                    xt, wt = dma_xw(t)
                    for rp in range(HALF):
                        first = (t == 0 and rp == 0)
                        last = (t == NT - 1 and rp == HALF - 1)
                        for h in range(2):
                            nc.tensor.matmul(
                                s1[:, h * 512:(h + 1) * 512],
                                xt[:, rp * B:(rp + 1) * B],
                                wt[:, rp * CM + h * 512:rp * CM + (h + 1) * 512],
                                start=first, stop=last,
                            )
                s_acc = sm.tile([B, CM], f32, tag="s_acc")
                nc.vector.tensor_copy(s_acc[:], s1[:])
                return [s_acc]

            def emit_iterN():
                s_accs = [sm.tile([B, CM], f32, tag="s_acc", name="s_accA")]
                if SPLIT_AR:
                    s_accs.append(sm.tile([B, CM], f32, tag="s_acc2", name="s_accB"))
                for t in range(NT):
                    s_acc = s_accs[-1] if (SPLIT_AR and t >= NT // 2) \
                        else s_accs[0]
                    t_first = t == 0 or (SPLIT_AR and t == NT // 2)
                    xt, wt = dma_xw(t)
                    ut = up.tile([B, RT * CM], f16)
                    for j in range(RT):
                        rp, par = j // 2, j % 2
                        ps = pp.tile([B, CM], f32, name="ps")
                        for h in range(2):
                            nc.tensor.matmul(
                                ps[:, h * 512:(h + 1) * 512],
                                xt[par * K:(par + 1) * K,
                                   rp * B:(rp + 1) * B],
                                wt[par * K:(par + 1) * K,
                                   rp * CM + h * 512:rp * CM + (h + 1) * 512],
                                start=True, stop=True,
                            )
                        dst = ut[:, j * CM:(j + 1) * CM]
                        use_act = (CPENG == "act" or
                                   (CPENG == "split" and j % 2 == 0))
                        if use_act:
                            nc.scalar.copy(dst, ps[:])
                        else:
                            nc.vector.tensor_copy(dst, ps[:])

                    if ABL == "prod":
                        nc.vector.tensor_copy(s_acc[:], ut[:, :CM])
                        continue

                    dot = None
                    if ABL not in ("nodot",):
                        # ---- dot = sum_m u * vbar (mult + m-halving tree) --
                        pt = pq.tile([B, RT * CM], f16, tag="pq")
                        nc.vector.tensor_tensor(
                            pt[:], ut[:],
                            _view(vbar16[:], [(0, RT), (1, CM)]), op=op.mult)

                        def tree_level(buf, half, out_ap=None):
                            a = _view(buf[:], [(CM, RT), (C, half), (1, C)])
                            import concourse.bass as bass
                            b2 = bass.AP(tensor=a.tensor,
                                         offset=a.offset + half * C,
                                         ap=[list(d) for d in a.ap])
                            o = out_ap if out_ap is not None else a
                            nc.vector.tensor_tensor(o, a, b2, op=op.add)

                        dot = sm.tile([B, RT * C], f16, tag="dot")
                        for half in (16, 8, 4, 2):
                            tree_level(pt, half)
                        tree_level(pt, 1,
                                   out_ap=_view(dot[:], [(C, RT), (1, C)]))

                    if ABL in ("full",) and dot is not None:
                        # ---- softmax over c (innermost) ----
                        mx = sm.tile([B, RT], f32, tag="mx")
                        nc.vector.tensor_reduce(
                            mx[:], _view(dot[:], [(C, RT), (1, C)]),
                            axis=AX.X, op=op.max)
                        e = sm.tile([B, RT * C], f16, tag="e")
                        nc.vector.tensor_tensor(
                            _view(e[:], [(C, RT), (1, C)]),
                            _view(dot[:], [(C, RT), (1, C)]),
                            _view(mx[:], [(1, RT), (0, C)]),
                            op=op.subtract)
                        nc.scalar.activation(
                            e[:], e[:], mybir.ActivationFunctionType.Exp)
                        z = sm.tile([B, RT], f32, tag="z")
                        nc.vector.tensor_reduce(
                            z[:], _view(e[:], [(C, RT), (1, C)]),
                            axis=AX.X, op=op.add)
                        nc.vector.reciprocal(z[:], z[:])
                        coef = sm.tile([B, RT * C], f16, tag="coef")
                        nc.vector.tensor_tensor(
                            _view(coef[:], [(C, RT), (1, C)]),
                            _view(e[:], [(C, RT), (1, C)]),
                            _view(z[:], [(1, RT), (0, C)]),
                            op=op.mult)
                    elif ABL == "nosm" and dot is not None:
                        coef = dot
                    else:  # nodot: dummy coefficients
                        coef = sm.tile([B, RT * C], f16, tag="coef")
                        nc.vector.memset(coef[:], 0.03125)

                    if ABL == "noq":
                        nc.vector.tensor_copy(s_acc[:], ut[:, :CM])
                        continue

                    # ---- q = u * coef ; s-tile = sum_r q (r-halving tree) --
                    qt = pq.tile([B, RT * CM], f16, tag="pq")
                    q_eng = (nc.gpsimd if (GPS and t % GPS == GPS - 1)
                             else nc.vector)
                    q_eng.tensor_tensor(
                        qt[:], ut[:],
                        _view(coef[:], [(C, RT), (0, M), (1, C)]),
                        op=op.mult)

                    half = RT // 2
                    while half >= 1:
                        a = qt[:, :half * CM]
                        b2 = qt[:, half * CM:2 * half * CM]
                        if half == 1:
                            if t_first:
                                nc.vector.tensor_tensor(
                                    s_acc[:], a, b2, op=op.add)
                            else:
                                stile = sm.tile([B, CM], f16, tag="stile")
                                nc.vector.tensor_tensor(
                                    stile[:], a, b2, op=op.add)
                                nc.vector.tensor_tensor(
                                    s_acc[:], s_acc[:], stile[:], op=op.add)
                        else:
                            nc.vector.tensor_tensor(a, a, b2, op=op.add)
                        half //= 2
                return s_accs

            for rep in range(repeat):
                with nc.named_scope("iter1"):
                    s_acc = emit_iter1()
                with nc.named_scope("ar1"):
                    allreduce_squash(s_acc, 1.0 / C, first=True)
                for it in range(2, n_iters + 1):
                    with nc.named_scope(f"iter{it}"):
                        s_acc = emit_iterN()
                    with nc.named_scope(f"ar{it}"):
                        allreduce_squash(s_acc, 1.0, first=False)

            nc.sync.dma_start(out[:], v_sb[:])

    if os.environ.get("DC_SKIP_COMPILE") != "1":
        nc.compile()
    return nc


def kernel(x, route_weights, num_iterations):
    global LAST_RESULT
    from concourse import bass_utils

    n = int(num_iterations)
    assert n >= 1
    x = np.asarray(x, dtype=np.float32)
    w = np.asarray(route_weights, dtype=np.float32)
    assert x.shape == (B, R, K) and w.shape == (R, C, K, M)

    if n not in _compiled:
        _compiled[n] = _build(n)
    nc = _compiled[n]

    in_maps = []
    for c in range(N_CORES):
        sl = slice(c * R_LOC, (c + 1) * R_LOC)
        xT_c = np.ascontiguousarray(
            x[:, sl, :].transpose(1, 2, 0).reshape(R_LOC // 2, 2, K, B)
        ).astype(np.float16)
        wT_c = np.ascontiguousarray(
            w[sl].reshape(R_LOC // 2, 2, C, K, M).transpose(0, 1, 3, 4, 2)
        ).reshape(R_LOC // 2, 2, K, CM).astype(np.float16)
        in_maps.append({"xT": xT_c, "wT": wT_c})

    res = bass_utils.run_bass_kernel_spmd(
        nc, in_maps, core_ids=list(range(N_CORES)))
    LAST_RESULT = res
    return np.ascontiguousarray(
        res.results[0]["out"].reshape(B, M, C).transpose(0, 2, 1)
    ).astype(np.float32)


# revision 23
# speedup vs baseline: 1.7747x; 1.7747x over previous
"""DigitCaps dynamic-routing kernel for 8 Trainium2 NeuronCores.

Strategy: shard num_route_nodes (R=2048 -> 256/core); AllReduce the s
contraction. No u staging: u is recomputed on the PE each routing pass
(w re-DMA in fp16 is cheaper than staging u to DRAM, and the PE is
otherwise idle).

  - Iteration 1 (uniform coefficients): s1 = sum_r u_r accumulated
    entirely on the PE via route-pair-packed K=128 matmuls. No vector
    work at all.
  - Iterations i>=2: per r-tile, recompute u (K=64 matmuls, fp16 in,
    fp32 PSUM), ACT copies PSUM->SBUF fp16. Routing logits use the
    linearity trick  b_i = dot(u, v_1+...+v_{i-1})  so there is no
    persistent logit tensor. All heavy vector ops run on the DVE in
    fp16 2x mode: products as tensor_tensor, reductions as binary
    add-trees (tensor_reduce is 1x-only; trees are 2x => ~2x faster).
  - CM layout is m-major (cm = m*C + c) so every broadcast/tree view
    has unit innermost stride (2x eligible): dot tree halves m, the
    coefficient broadcast repeats over m with c contiguous.

Host-side prep is layout/dtype only (transpose + fp16 cast); all FLOPs
on device.
"""

import os
import sys

if "/opt/trn_rl_repo" not in sys.path:
    sys.path.insert(0, "/opt/trn_rl_repo")

import numpy as np

B, R, K, C, M = 128, 2048, 64, 32, 32
CM = C * M
N_CORES = 8
R_LOC = R // N_CORES
RT = int(os.environ.get("DC_RT", "16"))     # routes per tile (even)
SIM_MODE = os.environ.get("DC_SIM", "0") == "1"   # 1-core, collective->copy
# Ablation knob (timing experiments only; wrong numerics except "full"):
#   full | nosm (skip softmax) | nodot (skip p/tree/softmax) |
#   noq (skip q/s-tree) | prod (u production only) | fakear (no collective)
ABL = os.environ.get("DC_ABL", "full")
# If >0: route every GPS-th tile's q-mult to GPSIMD (offload experiment)
GPS = int(os.environ.get("DC_GPS", "0"))
# Split each routing iteration's AllReduce in two so the first half's
# collective overlaps the second half's compute.
SPLIT_AR = os.environ.get("DC_SPLITAR", "0") == "1"
# PSUM->SBUF u-copy engine: act | split | dve
CPENG = os.environ.get("DC_CPENG", "act")

_compiled = {}
LAST_RESULT = None


def _view(ap, dims):
    """Free-dim view of an AP: keep its partition dim, replace free dims by
    [step, count] pairs (element steps). step 0 = broadcast."""
    import concourse.bass as bass

    return bass.AP(
        tensor=ap.tensor,
        offset=ap.offset,
        ap=[list(ap.ap[0])] + [[s, c] for s, c in dims],
    )


def _ap(ap, dims):
    """Fully custom AP (all dims given) at the base offset of `ap`."""
    import concourse.bass as bass

    return bass.AP(
        tensor=ap.tensor,
        offset=ap.offset,
        ap=[[s, c] for s, c in dims],
    )


def _squash(nc, pool, s_ap, v_ap):
    """v = s * |s|^2 / ((1 + |s|^2) (sqrt(|s|^2) + 1e-8)), norm over m.
    m-major layout: sq[b,c] reduces view [(1,C),(C,M)]."""
    import concourse.mybir as mybir

    f32 = mybir.dt.float32
    op = mybir.AluOpType
    sq_full = pool.tile([B, CM], f32, tag="sq_full")
    nc.vector.tensor_tensor(sq_full[:], s_ap, s_ap, op=op.mult)
    sq = pool.tile([B, C], f32, tag="sq")
    nc.vector.tensor_reduce(
        sq[:], _view(sq_full[:], [(1, C), (C, M)]), axis=mybir.AxisListType.X,
        op=op.add)
    rt = pool.tile([B, C], f32, tag="rt")
    nc.scalar.activation(rt[:], sq[:], mybir.ActivationFunctionType.Sqrt)
    nc.vector.tensor_scalar(rt[:], rt[:], 1e-8, None, op0=op.add)
    den = pool.tile([B, C], f32, tag="den")
    nc.vector.tensor_scalar(den[:], sq[:], 1.0, None, op0=op.add)
    nc.vector.tensor_tensor(den[:], den[:], rt[:], op=op.mult)
    fi = pool.tile([B, C], f32, tag="fi")
    nc.vector.reciprocal(fi[:], den[:])
    nc.vector.tensor_tensor(fi[:], fi[:], sq[:], op=op.mult)
    nc.vector.tensor_tensor(
        v_ap,
        _view(s_ap, [(C, M), (1, C)]),
        _view(fi[:], [(0, M), (1, C)]),
        op=op.mult,
    )


def _build(n_iters, repeat=1):
    import concourse.mybir as mybir
    import concourse.tile as tile
    from concourse import bacc

    f32 = mybir.dt.float32
    f16 = mybir.dt.float16
    op = mybir.AluOpType
    AX = mybir.AxisListType
    HALF = RT // 2
    NT = R_LOC // RT

    nc = bacc.Bacc("TRN2", target_bir_lowering=False, debug=False,
                   num_devices=1 if SIM_MODE else N_CORES)
    xT = nc.dram_tensor("xT", [R_LOC // 2, 2, K, B], f16,
                        kind="ExternalInput").ap()
    wT = nc.dram_tensor("wT", [R_LOC // 2, 2, K, CM], f16,
                        kind="ExternalInput").ap()
    out = nc.dram_tensor("out", [B, CM], f32, kind="ExternalOutput").ap()

    with tile.TileContext(nc) as tc:
        with (
            tc.tile_pool(name="sm", bufs=2) as sm,
            tc.tile_pool(name="persist", bufs=1) as persist,
            tc.tile_pool(name="xp", bufs=3) as xp,
            tc.tile_pool(name="wp", bufs=2) as wp,
            tc.tile_pool(name="up", bufs=2) as up,
            tc.tile_pool(name="pq", bufs=1) as pq,
            tc.tile_pool(name="pp", bufs=3, space="PSUM") as pp,
            tc.tile_pool(name="s1p", bufs=1, space="PSUM") as s1p,
            tc.tile_pool(name="drbounce", bufs=min(2 * n_iters * repeat, 8),
                         space="DRAM") as drb,
        ):
            v_sb = persist.tile([B, CM], f32)        # current v (output)
            vbar32 = persist.tile([B, CM], f32)      # sum of v's (logit trick)
            vbar16 = persist.tile([B, CM], f16)

            def dma_xw(t):
                """DMA one tile of x/w (RT routes = HALF pairs), packed
                [2K, HALF*B] / [2K, HALF*CM] (pair on partitions)."""
                xt = xp.tile([2 * K, HALF * B], f16)
                nc.sync.dma_start(
                    xt[:],
                    _ap(xT[t * HALF:(t + 1) * HALF],
                        [(B, 2 * K), (2 * K * B, HALF), (1, B)]))
                wt = wp.tile([2 * K, HALF * CM], f16)
                nc.sync.dma_start(
                    wt[:],
                    _ap(wT[t * HALF:(t + 1) * HALF],
                        [(CM, 2 * K), (2 * K * CM, HALF), (1, CM)]))
                return xt, wt

            def ar_start(s_acc_tile, tag):
                bin_ = drb.tile([B, CM], f32, tag=f"bin{tag}", name=f"bin{tag}")
                bout = drb.tile([B, CM], f32, tag=f"bout{tag}", name=f"bout{tag}")
                nc.sync.dma_start(bin_[:], s_acc_tile[:])
                if SIM_MODE or ABL == "fakear":
                    nc.sync.dma_start(bout[:], bin_[:])
                else:
                    nc.gpsimd.collective_compute(
                        "AllReduce", op.add,
                        replica_groups=[list(range(N_CORES))],
                        ins=[bin_.opt()], outs=[bout.opt()],
                    )
                return bout

            def allreduce_squash(s_accs, scale, first):
                bouts = [ar_start(s, i) for i, s in enumerate(s_accs)]
                s_sb = sm.tile([B, CM], f32, tag="s_sb")
                nc.sync.dma_start(s_sb[:], bouts[0][:])
                for bout in bouts[1:]:
                    more = sm.tile([B, CM], f32, tag="s_sb2")
                    nc.sync.dma_start(more[:], bout[:])
                    nc.vector.tensor_tensor(s_sb[:], s_sb[:], more[:],
                                            op=op.add)
                if scale != 1.0:
                    nc.vector.tensor_scalar(s_sb[:], s_sb[:], scale, None,
                                            op0=op.mult)
                _squash(nc, sm, s_sb[:], v_sb[:])
                if first:
                    nc.vector.tensor_copy(vbar32[:], v_sb[:])
                else:
                    nc.vector.tensor_tensor(vbar32[:], vbar32[:], v_sb[:],
                                            op=op.add)
                nc.vector.tensor_copy(vbar16[:], vbar32[:])

            def emit_iter1():
                """s1 = sum_r u_r entirely on PE: packed K=128 matmuls."""
                s1 = s1p.tile([B, CM], f32, name="s1")
                for t in range(NT):
                    xt, wt = dma_xw(t)
                    for rp in range(HALF):
                        first = (t == 0 and rp == 0)
                        last = (t == NT - 1 and rp == HALF - 1)
                        for h in range(2):
                            nc.tensor.matmul(
                                s1[:, h * 512:(h + 1) * 512],
                                xt[:, rp * B:(rp + 1) * B],
                                wt[:, rp * CM + h * 512:rp * CM + (h + 1) * 512],
                                start=first, stop=last,
                            )
                s_acc = sm.tile([B, CM], f32, tag="s_acc")
                nc.vector.tensor_copy(s_acc[:], s1[:])
                return [s_acc]

            def emit_iterN():
                s_accs = [sm.tile([B, CM], f32, tag="s_acc", name="s_accA")]
                if SPLIT_AR:
                    s_accs.append(sm.tile([B, CM], f32, tag="s_acc2", name="s_accB"))
                for t in range(NT):
                    s_acc = s_accs[-1] if (SPLIT_AR and t >= NT // 2) \
                        else s_accs[0]
                    t_first = t == 0 or (SPLIT_AR and t == NT // 2)
                    xt, wt = dma_xw(t)
                    ut = up.tile([B, RT * CM], f16)
                    for j in range(RT):
                        rp, par = j // 2, j % 2
                        ps = pp.tile([B, CM], f32, name="ps")
                        for h in range(2):
                            nc.tensor.matmul(
                                ps[:, h * 512:(h + 1) * 512],
                                xt[par * K:(par + 1) * K,
                                   rp * B:(rp + 1) * B],
                                wt[par * K:(par + 1) * K,
                                   rp * CM + h * 512:rp * CM + (h + 1) * 512],
                                start=True, stop=True,
                            )
                        dst = ut[:, j * CM:(j + 1) * CM]
                        use_act = (CPENG == "act" or
                                   (CPENG == "split" and j % 2 == 0))
                        if use_act:
                            nc.scalar.copy(dst, ps[:])
                        else:
                            nc.vector.tensor_copy(dst, ps[:])

                    if ABL == "prod":
                        nc.vector.tensor_copy(s_acc[:], ut[:, :CM])
                        continue

                    dot = None
                    if ABL not in ("nodot",):
                        # ---- dot = sum_m u * vbar (mult + m-halving tree) --
                        pt = pq.tile([B, RT * CM], f16, tag="pq")
                        nc.vector.tensor_tensor(
                            pt[:], ut[:],
                            _view(vbar16[:], [(0, RT), (1, CM)]), op=op.mult)

                        def tree_level(buf, half, out_ap=None):
                            a = _view(buf[:], [(CM, RT), (C, half), (1, C)])
                            import concourse.bass as bass
                            b2 = bass.AP(tensor=a.tensor,
                                         offset=a.offset + half * C,
                                         ap=[list(d) for d in a.ap])
                            o = out_ap if out_ap is not None else a
                            nc.vector.tensor_tensor(o, a, b2, op=op.add)

                        dot = sm.tile([B, RT * C], f16, tag="dot")
                        for half in (16, 8, 4, 2):
                            tree_level(pt, half)
                        tree_level(pt, 1,
                                   out_ap=_view(dot[:], [(C, RT), (1, C)]))

                    if ABL in ("full",) and dot is not None:
                        # ---- softmax over c (innermost) ----
                        mx = sm.tile([B, RT], f32, tag="mx")
                        nc.vector.tensor_reduce(
                            mx[:], _view(dot[:], [(C, RT), (1, C)]),
                            axis=AX.X, op=op.max)
                        e = sm.tile([B, RT * C], f16, tag="e")
                        nc.vector.tensor_tensor(
                            _view(e[:], [(C, RT), (1, C)]),
                            _view(dot[:], [(C, RT), (1, C)]),
                            _view(mx[:], [(1, RT), (0, C)]),
                            op=op.subtract)
                        nc.scalar.activation(
                            e[:], e[:], mybir.ActivationFunctionType.Exp)
                        z = sm.tile([B, RT], f32, tag="z")
                        nc.vector.tensor_reduce(
                            z[:], _view(e[:], [(C, RT), (1, C)]),
                            axis=AX.X, op=op.add)
                        nc.vector.reciprocal(z[:], z[:])
                        coef = sm.tile([B, RT * C], f16, tag="coef")
                        nc.vector.tensor_tensor(
                            _view(coef[:], [(C, RT), (1, C)]),
                            _view(e[:], [(C, RT), (1, C)]),
                            _view(z[:], [(1, RT), (0, C)]),
                            op=op.mult)
                    elif ABL == "nosm" and dot is not None:
                        coef = dot
                    else:  # nodot: dummy coefficients
                        coef = sm.tile([B, RT * C], f16, tag="coef")
                        nc.vector.memset(coef[:], 0.03125)

                    if ABL == "noq":
                        nc.vector.tensor_copy(s_acc[:], ut[:, :CM])
                        continue

                    # ---- q = u * coef ; s-tile = sum_r q (r-halving tree) --
                    qt = pq.tile([B, RT * CM], f16, tag="pq")
                    q_eng = (nc.gpsimd if (GPS and t % GPS == GPS - 1)
                             else nc.vector)
                    q_eng.tensor_tensor(
                        _view(qt[:], [(CM, RT), (C, M), (1, C)]),
                        _view(ut[:], [(CM, RT), (C, M), (1, C)]),
                        _view(coef[:], [(C, RT), (0, M), (1, C)]),
                        op=op.mult)

                    import concourse.bass as bass
                    half = RT // 2
                    while half >= 1:
                        a = _view(qt[:], [(CM, half), (1, CM)])
                        b2 = bass.AP(tensor=a.tensor,
                                     offset=a.offset + half * CM,
                                     ap=[list(d) for d in a.ap])
                        if half == 1:
                            if t_first:
                                nc.vector.tensor_tensor(
                                    s_acc[:], a, b2, op=op.add)
                            else:
                                stile = sm.tile([B, CM], f16, tag="stile")
                                nc.vector.tensor_tensor(
                                    stile[:], a, b2, op=op.add)
                                nc.vector.tensor_tensor(
                                    s_acc[:], s_acc[:], stile[:], op=op.add)
                        else:
                            nc.vector.tensor_tensor(a, a, b2, op=op.add)
                        half //= 2
                return s_accs

            for rep in range(repeat):
                with nc.named_scope("iter1"):
                    s_acc = emit_iter1()
                with nc.named_scope("ar1"):
                    allreduce_squash(s_acc, 1.0 / C, first=True)
                for it in range(2, n_iters + 1):
                    with nc.named_scope(f"iter{it}"):
                        s_acc = emit_iterN()
                    with nc.named_scope(f"ar{it}"):
                        allreduce_squash(s_acc, 1.0, first=False)

            nc.sync.dma_start(out[:], v_sb[:])

    if os.environ.get("DC_SKIP_COMPILE") != "1":
        nc.compile()
    return nc


def kernel(x, route_weights, num_iterations):
    global LAST_RESULT
    from concourse import bass_utils

    n = int(num_iterations)
    assert n >= 1
    x = np.asarray(x, dtype=np.float32)
    w = np.asarray(route_weights, dtype=np.float32)
    assert x.shape == (B, R, K) and w.shape == (R, C, K, M)

    if n not in _compiled:
        _compiled[n] = _build(n)
    nc = _compiled[n]

    in_maps = []
    for c in range(N_CORES):
        sl = slice(c * R_LOC, (c + 1) * R_LOC)
        xT_c = np.ascontiguousarray(
            x[:, sl, :].transpose(1, 2, 0).reshape(R_LOC // 2, 2, K, B)
        ).astype(np.float16)
        wT_c = np.ascontiguousarray(
            w[sl].reshape(R_LOC // 2, 2, C, K, M).transpose(0, 1, 3, 4, 2)
        ).reshape(R_LOC // 2, 2, K, CM).astype(np.float16)
        in_maps.append({"xT": xT_c, "wT": wT_c})

    res = bass_utils.run_bass_kernel_spmd(
        nc, in_maps, core_ids=list(range(N_CORES)))
    LAST_RESULT = res
    return np.ascontiguousarray(
        res.results[0]["out"].reshape(B, M, C).transpose(0, 2, 1)
    ).astype(np.float32)
